# revision 19
# baseline (speedup 1.0000x reference)
"""Trainium2 8-core kernel for nn_AdvancedEmbeddingBlock.

Reference computation:
    x_phys = props[x] @ phys_w + phys_b
    x_ = concat(emb_w[x], tag_w[tag], x_phys, period_w[period_idx[x]], group_w[group_idx[x]])
    rbf_h = swish(rbf @ rbf_w + rbf_b)
    out = swish(concat(x_[i], x_[j], rbf_h) @ lin_w + lin_b)        # [E, 256]

Algebraic rewrite used here: with lin_w = [W1; W2; W3] (256 rows each),
    out = swish(A[i] + B[j] + rbf_h @ W3 + lin_b),  A = x_ @ W1,  B = x_ @ W2
and x_[n] depends only on (x[n], tag[n]), which factors further into
85-row + 3-row tables:
    A[i] = AU[x[i]] + AT[tag[i]],   B[j] = BU[x[j]] + BT[tag[j]]
The per-edge gathers are then tiny-table lookups, realized on the
TensorEngine as one-hot (multi-hot) matmuls; the multi-hot masks are
encoded on the host directly from the index tensors. Everything on
device is dense bf16 GEMM + Silu: no indirect addressing, no collectives.

Per core (edges sharded 8 ways, all tables replicated):
    rbf_h^T = silu(RW.T @ rbfT)            RW = [rbf_w; rbf_b] (65x256), rbfT has a ones row
    p2      = T2.T @ mh1 + T3.T @ mh2 + W3.T @ rbf_h^T
    out^T   = silu(p2)                      (bf16, transposed back on host)
"""

import numpy as np
import ml_dtypes

import concourse.bass as bass  # noqa: F401  (engine types)
import concourse.mybir as mybir
import concourse.tile as tile
from concourse import bacc
from concourse.bass_utils import run_bass_kernel_spmd

BF16 = mybir.dt.bfloat16
FP8 = mybir.dt.float8e4
FP32 = mybir.dt.float32
AFT = mybir.ActivationFunctionType

N_CORES = 8
N_EDGES = 400000
HID = 256
RAD = 64
ET = 512                      # edges per compute subtile
MACRO = 7168                  # edges per DMA macro-tile (= 14 * 512)
NSUB = MACRO // ET
NPAIR = NSUB // 2             # subtile pairs per macro
E_LOC = 50176                 # padded edges per core (= 7 * 7168)
NM = E_LOC // MACRO
E_PAD = N_CORES * E_LOC       # 401408

_CACHE = {}


def _build_nc():
    """Build the SPMD Bass graph (identical on all 8 cores)."""
    nc = bacc.Bacc("TRN2", target_bir_lowering=False, debug=False,
                   enable_asserts=False, num_devices=N_CORES)
    rbfT = nc.dram_tensor("rbfT", [65, E_LOC], BF16, kind="ExternalInput")
    mh1 = nc.dram_tensor("mh1", [91, E_LOC], FP8, kind="ExternalInput")
    mh2 = nc.dram_tensor("mh2", [85, E_LOC], FP8, kind="ExternalInput")
    cpk = nc.dram_tensor("cpk", [128, 5 * HID], BF16, kind="ExternalInput")
    outT = nc.dram_tensor("outT", [HID, E_LOC], BF16, kind="ExternalOutput")

    with tile.TileContext(nc) as tc:
        with (
            tc.tile_pool(name="consts", bufs=1) as consts,
            tc.tile_pool(name="io", bufs=2) as io,
            tc.tile_pool(name="om", bufs=2) as om,
            tc.tile_pool(name="mid", bufs=3) as mid,
            tc.tile_pool(name="ps1", bufs=1, space="PSUM") as ps1,
            tc.tile_pool(name="ps2", bufs=2, space="PSUM") as ps2,
        ):
            # one packed const load: [w3a | w3b | rw | t2 | t3]
            ct = consts.tile([128, 5 * HID], BF16)
            nc.sync.dma_start(ct[:], cpk[:])
            w3a_t = ct[:, 0 * HID:1 * HID]
            w3b_t = ct[:, 1 * HID:2 * HID]
            rw_t = ct[0:65, 2 * HID:3 * HID]
            t2_t = ct[0:91, 3 * HID:4 * HID]
            t3_t = ct[0:85, 4 * HID:5 * HID]

            # touch the Silu table early so ACT_TABLE_LOAD overlaps the
            # first input DMAs instead of sitting on the critical path
            scr = consts.tile([128, 8], BF16)
            nc.vector.memset(scr[:], 0.0)
            scr2 = consts.tile([128, 8], BF16)
            nc.scalar.activation(scr2[:], scr[:], AFT.Silu)

            f0, f1 = slice(0, 128), slice(128, 256)
            for m in range(NM):
                ms = slice(m * MACRO, (m + 1) * MACRO)
                rbf_m = io.tile([65, MACRO], BF16, tag="rbf")
                mh1_m = io.tile([91, MACRO], FP8, tag="mh1")
                mh2_m = io.tile([85, MACRO], FP8, tag="mh2")
                if m == 0:
                    # chunk the first macro's loads so compute starts early
                    for p in range(NPAIR):
                        cs = slice(2 * p * ET, 2 * (p + 1) * ET)
                        nc.sync.dma_start(rbf_m[:, cs], rbfT[:, cs])
                        nc.sync.dma_start(mh1_m[:, cs], mh1[:, cs])
                        nc.sync.dma_start(mh2_m[:, cs], mh2[:, cs])
                else:
                    nc.sync.dma_start(rbf_m[:], rbfT[:, ms])
                    nc.sync.dma_start(mh1_m[:], mh1[:, ms])
                    nc.sync.dma_start(mh2_m[:], mh2[:, ms])
                # output staging: f0-half [0:MACRO], f1-half [MACRO:2*MACRO]
                out_m = om.tile([128, 2 * MACRO], BF16, tag="out_m")

                for p in range(NPAIR):
                    es0 = slice((2 * p) * ET, (2 * p + 1) * ET)
                    es1 = slice((2 * p + 1) * ET, (2 * p + 2) * ET)
                    # GEMM1 for both subtiles of the pair, weights loaded once
                    p1 = ps1.tile([128, 4 * ET], FP32, tag="p1")
                    nc.tensor.matmul(p1[:, 0:ET], rw_t[:, f0], rbf_m[:, es0],
                                     start=True, stop=True)
                    nc.tensor.matmul(p1[:, 2 * ET:3 * ET], rw_t[:, f0],
                                     rbf_m[:, es1], start=True, stop=True)
                    nc.tensor.matmul(p1[:, ET:2 * ET], rw_t[:, f1],
                                     rbf_m[:, es0], start=True, stop=True)
                    nc.tensor.matmul(p1[:, 3 * ET:4 * ET], rw_t[:, f1],
                                     rbf_m[:, es1], start=True, stop=True)
                    s1 = mid.tile([128, 4 * ET], BF16, tag="s1")
                    nc.scalar.activation(s1[:], p1[:], AFT.Silu)

                    # p2 = T2.T@mh1 + T3.T@mh2 + W3.T@s1  (bias folded in T2);
                    # gathers first (no s1 dep), W3 last; each weight loaded
                    # once per pair
                    pA = ps2.tile([128, 2 * ET], FP32, tag="p2")
                    pB = ps2.tile([128, 2 * ET], FP32, tag="p2")
                    for (w, h, rha, rhb, st, sp) in (
                        (t2_t[:, f0], 0, mh1_m[:, es0], mh1_m[:, es1], True, False),
                        (t2_t[:, f1], ET, mh1_m[:, es0], mh1_m[:, es1], True, False),
                        (t3_t[:, f0], 0, mh2_m[:, es0], mh2_m[:, es1], False, False),
                        (t3_t[:, f1], ET, mh2_m[:, es0], mh2_m[:, es1], False, False),
                        (w3a_t[:, f0], 0, s1[:, 0:ET], s1[:, 2 * ET:3 * ET], False, False),
                        (w3a_t[:, f1], ET, s1[:, 0:ET], s1[:, 2 * ET:3 * ET], False, False),
                        (w3b_t[:, f0], 0, s1[:, ET:2 * ET], s1[:, 3 * ET:4 * ET], False, True),
                        (w3b_t[:, f1], ET, s1[:, ET:2 * ET], s1[:, 3 * ET:4 * ET], False, True),
                    ):
                        nc.tensor.matmul(pA[:, h:h + ET], w, rha,
                                         start=st, stop=sp)
                        nc.tensor.matmul(pB[:, h:h + ET], w, rhb,
                                         start=st, stop=sp)
                    for (pt, es) in ((pA, es0), (pB, es1)):
                        dst = out_m[:].rearrange("p (f e) -> p f e", f=2)[:, :, es]
                        nc.scalar.activation(
                            dst, pt[:].rearrange("p (f e) -> p f e", f=2), AFT.Silu)

                # store macro; last macro stores in pair chunks for a short tail
                if m == NM - 1:
                    for p in range(NPAIR):
                        cs = slice(m * MACRO + 2 * p * ET, m * MACRO + 2 * (p + 1) * ET)
                        ls = slice(2 * p * ET, 2 * (p + 1) * ET)
                        nc.sync.dma_start(outT[0:128, cs], out_m[:, ls])
                        nc.sync.dma_start(
                            outT[128:256, cs],
                            out_m[:, MACRO + 2 * p * ET:MACRO + 2 * (p + 1) * ET])
                else:
                    nc.sync.dma_start(outT[0:128, ms], out_m[:, 0:MACRO])
                    nc.sync.dma_start(outT[128:256, ms], out_m[:, MACRO:2 * MACRO])
    nc.compile()
    return nc


def get_nc():
    if "nc" not in _CACHE:
        _CACHE["nc"] = _build_nc()
    return _CACHE["nc"]


def prepare(x, tag, i, j, rbf, period_idx, group_idx, props,
            emb_w, tag_w, period_w, group_w, phys_w, phys_b,
            rbf_w, rbf_b, lin_w, lin_b):
    """Host-side staging: tables, multi-hot index encodings, sharding."""
    bf = ml_dtypes.bfloat16
    f32 = np.float32
    x = np.asarray(x).astype(np.int64)
    tag = np.asarray(tag).astype(np.int64)
    i = np.asarray(i).astype(np.int64)
    j = np.asarray(j).astype(np.int64)

    # 85-row / 3-row node tables
    x_phys = props.astype(f32) @ phys_w.astype(f32) + phys_b.astype(f32)
    U85 = np.zeros((85, HID), f32)
    U85[:, 0:128] = emb_w
    U85[:, 160:192] = x_phys
    U85[:, 192:224] = period_w[np.asarray(period_idx).astype(np.int64)]
    U85[:, 224:256] = group_w[np.asarray(group_idx).astype(np.int64)]
    Utag = np.zeros((3, HID), f32)
    Utag[:, 128:160] = tag_w
    W1 = lin_w[0:256].astype(f32)
    W2 = lin_w[256:512].astype(f32)
    W3 = lin_w[512:768].astype(f32)
    AU = U85 @ W1 + lin_b.astype(f32)      # fold bias into the A-table
    AT = Utag @ W1
    BU = U85 @ W2
    BT = Utag @ W2
    T2 = np.concatenate([AU, AT, BT], 0)                                    # [91, 256]
    T3 = BU                                                                 # [85, 256]
    RW = np.concatenate([rbf_w.astype(f32), rbf_b.astype(f32)[None, :]], 0)  # [65, 256]
    # packed const tensor: [w3a | w3b | rw | t2 | t3], each a 256-col block
    CPK = np.zeros((128, 5 * HID), f32)
    CPK[:, 0 * HID:1 * HID] = W3[0:128]
    CPK[:, 1 * HID:2 * HID] = W3[128:256]
    CPK[0:65, 2 * HID:3 * HID] = RW
    CPK[0:91, 3 * HID:4 * HID] = T2
    CPK[0:85, 4 * HID:5 * HID] = T3
    CPK = CPK.astype(bf)

    # multi-hot masks (fp8e4m3 one-hot: 1.0 == 0x38; cast to bf16 during DMA)
    ONE = np.uint8(0x38)
    ar = np.arange(N_EDGES)
    xi, ti_ = x[i], tag[i]
    xj, tj = x[j], tag[j]
    mh1 = np.zeros((91, E_PAD), np.uint8)
    mh1[xi, ar] = ONE
    mh1[85 + ti_, ar] = ONE
    mh1[88 + tj, ar] = ONE
    mh2 = np.zeros((85, E_PAD), np.uint8)
    mh2[xj, ar] = ONE
    mh1 = mh1.view(ml_dtypes.float8_e4m3fn)
    mh2 = mh2.view(ml_dtypes.float8_e4m3fn)

    rbfT = np.zeros((65, E_PAD), bf)
    rbfT[0:64, 0:N_EDGES] = rbf.astype(bf).T
    rbfT[64, 0:N_EDGES] = f32(1.0)

    in_maps = []
    for c in range(N_CORES):
        sl = slice(c * E_LOC, (c + 1) * E_LOC)
        in_maps.append(dict(
            rbfT=np.ascontiguousarray(rbfT[:, sl]),
            mh1=np.ascontiguousarray(mh1[:, sl]),
            mh2=np.ascontiguousarray(mh2[:, sl]),
            cpk=CPK,
        ))
    return in_maps


def unshard(results):
    out = np.empty((N_EDGES, HID), np.float32)
    for c in range(N_CORES):
        lo = c * E_LOC
        hi = min(lo + E_LOC, N_EDGES)
        blk = np.asarray(results[c]["outT"])[:, :hi - lo]
        out[lo:hi] = blk.astype(np.float32).T
    return out


def kernel(**inputs):
    in_maps = prepare(**inputs)
    nc = get_nc()
    res = run_bass_kernel_spmd(nc, in_maps, core_ids=list(range(N_CORES)))
    return unshard(res.results)


# revision 22
# speedup vs baseline: 1.2334x; 1.2334x over previous
"""Trainium2 8-core kernel for nn_AdvancedEmbeddingBlock.

Reference computation:
    x_phys = props[x] @ phys_w + phys_b
    x_ = concat(emb_w[x], tag_w[tag], x_phys, period_w[period_idx[x]], group_w[group_idx[x]])
    rbf_h = swish(rbf @ rbf_w + rbf_b)
    out = swish(concat(x_[i], x_[j], rbf_h) @ lin_w + lin_b)        # [E, 256]

Algebraic rewrite used here: with lin_w = [W1; W2; W3] (256 rows each),
    out = swish(A[i] + B[j] + rbf_h @ W3 + lin_b),  A = x_ @ W1,  B = x_ @ W2
and x_[n] depends only on (x[n], tag[n]), which factors further into
85-row + 3-row tables:
    A[i] = AU[x[i]] + AT[tag[i]],   B[j] = BU[x[j]] + BT[tag[j]]
The per-edge gathers are then tiny-table lookups, realized on the
TensorEngine as one-hot (multi-hot) matmuls; the multi-hot masks are
encoded on the host directly from the index tensors. Everything on
device is dense bf16 GEMM + Silu: no indirect addressing, no collectives.

Per core (edges sharded 8 ways, all tables replicated):
    rbf_h^T = silu(RW.T @ rbfT)            RW = [rbf_w; rbf_b] (65x256), rbfT has a ones row
    p2      = T2.T @ mh1 + T3.T @ mh2 + W3.T @ rbf_h^T
    out^T   = silu(p2)                      (bf16, transposed back on host)
"""

import numpy as np
import ml_dtypes

import concourse.bass as bass  # noqa: F401  (engine types)
import concourse.mybir as mybir
import concourse.tile as tile
from concourse import bacc
from concourse.bass_utils import run_bass_kernel_spmd

BF16 = mybir.dt.bfloat16
FP8 = mybir.dt.float8e4
FP32 = mybir.dt.float32
AFT = mybir.ActivationFunctionType

N_CORES = 8
N_EDGES = 400000
HID = 256
RAD = 64
ET = 512                      # edges per compute subtile
MACRO = 3584                  # edges per DMA macro-tile (= 7 * 512)
NSUB = MACRO // ET
E_LOC = 50176                 # padded edges per core (= 14 * 3584)
NM = E_LOC // MACRO
E_PAD = N_CORES * E_LOC       # 401408

_CACHE = {}


def _build_nc():
    """Build the SPMD Bass graph (identical on all 8 cores)."""
    nc = bacc.Bacc("TRN2", target_bir_lowering=False, debug=False,
                   enable_asserts=False, num_devices=N_CORES)
    rbfT = nc.dram_tensor("rbfT", [65, E_LOC], BF16, kind="ExternalInput")
    mh1 = nc.dram_tensor("mh1", [91, E_LOC], FP8, kind="ExternalInput")
    mh2 = nc.dram_tensor("mh2", [85, E_LOC], FP8, kind="ExternalInput")
    cpk = nc.dram_tensor("cpk", [128, 5 * HID], BF16, kind="ExternalInput")
    outT = nc.dram_tensor("outT", [HID, E_LOC], BF16, kind="ExternalOutput")

    with tile.TileContext(nc) as tc:
        with (
            tc.tile_pool(name="consts", bufs=1) as consts,
            tc.tile_pool(name="io", bufs=3) as io,
            tc.tile_pool(name="om", bufs=2) as om,
            tc.tile_pool(name="mid", bufs=3) as mid,
            tc.tile_pool(name="psum", bufs=2, space="PSUM") as psum,
        ):
            # one packed const load: [w3a | w3b | rw | t2 | t3]
            ct = consts.tile([128, 5 * HID], BF16)
            nc.sync.dma_start(ct[:], cpk[:])
            w3a_t = ct[:, 0 * HID:1 * HID]
            w3b_t = ct[:, 1 * HID:2 * HID]
            rw_t = ct[0:65, 2 * HID:3 * HID]
            t2_t = ct[0:91, 3 * HID:4 * HID]
            t3_t = ct[0:85, 4 * HID:5 * HID]

            # touch the Silu table early so ACT_TABLE_LOAD overlaps the
            # first input DMAs instead of sitting on the critical path
            scr = consts.tile([128, 8], BF16)
            nc.vector.memset(scr[:], 0.0)
            scr2 = consts.tile([128, 8], BF16)
            nc.scalar.activation(scr2[:], scr[:], AFT.Silu)

            f0, f1 = slice(0, 128), slice(128, 256)
            for m in range(NM):
                ms = slice(m * MACRO, (m + 1) * MACRO)
                rbf_m = io.tile([65, MACRO], BF16, tag="rbf")
                mh1_m = io.tile([91, MACRO], FP8, tag="mh1")
                mh2_m = io.tile([85, MACRO], FP8, tag="mh2")
                if m == 0:
                    # chunk the first macro's loads so compute starts early
                    for s in range(NSUB):
                        cs = slice(s * ET, (s + 1) * ET)
                        nc.sync.dma_start(rbf_m[:, cs], rbfT[:, cs])
                        nc.sync.dma_start(mh1_m[:, cs], mh1[:, cs])
                        nc.sync.dma_start(mh2_m[:, cs], mh2[:, cs])
                else:
                    nc.sync.dma_start(rbf_m[:], rbfT[:, ms])
                    nc.sync.dma_start(mh1_m[:], mh1[:, ms])
                    nc.sync.dma_start(mh2_m[:], mh2[:, ms])
                # output staging: f0-half [0:MACRO], f1-half [MACRO:2*MACRO]
                out_m = om.tile([128, 2 * MACRO], BF16, tag="out_m")

                for s in range(NSUB):
                    es = slice(s * ET, (s + 1) * ET)
                    # GEMM1: rbf_h^T (pre-activation), feature-major halves
                    p1 = psum.tile([128, 2 * ET], FP32, tag="p1")
                    nc.tensor.matmul(p1[:, 0:ET], rw_t[:, f0], rbf_m[:, es],
                                     start=True, stop=True)
                    nc.tensor.matmul(p1[:, ET:2 * ET], rw_t[:, f1],
                                     rbf_m[:, es], start=True, stop=True)
                    s1 = mid.tile([128, 2 * ET], BF16, tag="s1")
                    nc.scalar.activation(s1[:], p1[:], AFT.Silu)

                    # p2 = T2.T@mh1 + T3.T@mh2 + W3.T@s1  (bias folded into T2)
                    # gather MMs first (no s1 dependency), W3 MMs last so the
                    # PE never waits on silu1
                    p2 = psum.tile([128, 2 * ET], FP32, tag="p2")
                    pa, pb = p2[:, 0:ET], p2[:, ET:2 * ET]
                    nc.tensor.matmul(pa, t2_t[:, f0], mh1_m[:, es],
                                     start=True, stop=False)
                    nc.tensor.matmul(pb, t2_t[:, f1], mh1_m[:, es],
                                     start=True, stop=False)
                    nc.tensor.matmul(pa, t3_t[:, f0], mh2_m[:, es],
                                     start=False, stop=False)
                    nc.tensor.matmul(pb, t3_t[:, f1], mh2_m[:, es],
                                     start=False, stop=False)
                    nc.tensor.matmul(pa, w3a_t[:, f0], s1[:, 0:ET],
                                     start=False, stop=False)
                    nc.tensor.matmul(pb, w3a_t[:, f1], s1[:, 0:ET],
                                     start=False, stop=False)
                    nc.tensor.matmul(pa, w3b_t[:, f0], s1[:, ET:2 * ET],
                                     start=False, stop=True)
                    nc.tensor.matmul(pb, w3b_t[:, f1], s1[:, ET:2 * ET],
                                     start=False, stop=True)
                    dst = out_m[:].rearrange("p (f e) -> p f e", f=2)[:, :, es]
                    nc.scalar.activation(dst, p2[:].rearrange("p (f e) -> p f e", f=2),
                                         AFT.Silu)

                # store macro; last macro stores per-subtile for a short tail
                if m == NM - 1:
                    for s in range(NSUB):
                        cs = slice(m * MACRO + s * ET, m * MACRO + (s + 1) * ET)
                        nc.sync.dma_start(outT[0:128, cs], out_m[:, s * ET:(s + 1) * ET])
                        nc.sync.dma_start(
                            outT[128:256, cs],
                            out_m[:, MACRO + s * ET:MACRO + (s + 1) * ET])
                else:
                    nc.sync.dma_start(outT[0:128, ms], out_m[:, 0:MACRO])
                    nc.sync.dma_start(outT[128:256, ms], out_m[:, MACRO:2 * MACRO])
    nc.compile()
    return nc


def get_nc():
    if "nc" not in _CACHE:
        _CACHE["nc"] = _build_nc()
    return _CACHE["nc"]


def prepare(x, tag, i, j, rbf, period_idx, group_idx, props,
            emb_w, tag_w, period_w, group_w, phys_w, phys_b,
            rbf_w, rbf_b, lin_w, lin_b):
    """Host-side staging: tables, multi-hot index encodings, sharding."""
    bf = ml_dtypes.bfloat16
    f32 = np.float32
    x = np.asarray(x).astype(np.int64)
    tag = np.asarray(tag).astype(np.int64)
    i = np.asarray(i).astype(np.int64)
    j = np.asarray(j).astype(np.int64)

    # 85-row / 3-row node tables
    x_phys = props.astype(f32) @ phys_w.astype(f32) + phys_b.astype(f32)
    U85 = np.zeros((85, HID), f32)
    U85[:, 0:128] = emb_w
    U85[:, 160:192] = x_phys
    U85[:, 192:224] = period_w[np.asarray(period_idx).astype(np.int64)]
    U85[:, 224:256] = group_w[np.asarray(group_idx).astype(np.int64)]
    Utag = np.zeros((3, HID), f32)
    Utag[:, 128:160] = tag_w
    W1 = lin_w[0:256].astype(f32)
    W2 = lin_w[256:512].astype(f32)
    W3 = lin_w[512:768].astype(f32)
    AU = U85 @ W1 + lin_b.astype(f32)      # fold bias into the A-table
    AT = Utag @ W1
    BU = U85 @ W2
    BT = Utag @ W2
    T2 = np.concatenate([AU, AT, BT], 0)                                    # [91, 256]
    T3 = BU                                                                 # [85, 256]
    RW = np.concatenate([rbf_w.astype(f32), rbf_b.astype(f32)[None, :]], 0)  # [65, 256]
    # packed const tensor: [w3a | w3b | rw | t2 | t3], each a 256-col block
    CPK = np.zeros((128, 5 * HID), f32)
    CPK[:, 0 * HID:1 * HID] = W3[0:128]
    CPK[:, 1 * HID:2 * HID] = W3[128:256]
    CPK[0:65, 2 * HID:3 * HID] = RW
    CPK[0:91, 3 * HID:4 * HID] = T2
    CPK[0:85, 4 * HID:5 * HID] = T3
    CPK = CPK.astype(bf)

    # multi-hot masks (fp8e4m3 one-hot: 1.0 == 0x38; cast to bf16 during DMA)
    ONE = np.uint8(0x38)
    ar = np.arange(N_EDGES)
    xi, ti_ = x[i], tag[i]
    xj, tj = x[j], tag[j]
    mh1 = np.zeros((91, E_PAD), np.uint8)
    mh1[xi, ar] = ONE
    mh1[85 + ti_, ar] = ONE
    mh1[88 + tj, ar] = ONE
    mh2 = np.zeros((85, E_PAD), np.uint8)
    mh2[xj, ar] = ONE
    mh1 = mh1.view(ml_dtypes.float8_e4m3fn)
    mh2 = mh2.view(ml_dtypes.float8_e4m3fn)

    rbfT = np.zeros((65, E_PAD), bf)
    rbfT[0:64, 0:N_EDGES] = rbf.astype(bf).T
    rbfT[64, 0:N_EDGES] = f32(1.0)

    in_maps = []
    for c in range(N_CORES):
        sl = slice(c * E_LOC, (c + 1) * E_LOC)
        in_maps.append(dict(
            rbfT=np.ascontiguousarray(rbfT[:, sl]),
            mh1=np.ascontiguousarray(mh1[:, sl]),
            mh2=np.ascontiguousarray(mh2[:, sl]),
            cpk=CPK,
        ))
    return in_maps


def unshard(results):
    out = np.empty((N_EDGES, HID), np.float32)
    for c in range(N_CORES):
        lo = c * E_LOC
        hi = min(lo + E_LOC, N_EDGES)
        blk = np.asarray(results[c]["outT"])[:, :hi - lo]
        out[lo:hi] = blk.astype(np.float32).T
    return out


def kernel(**inputs):
    in_maps = prepare(**inputs)
    nc = get_nc()
    res = run_bass_kernel_spmd(nc, in_maps, core_ids=list(range(N_CORES)))
    return unshard(res.results)
